# revision 16
# baseline (speedup 1.0000x reference)
"""BitLinear forward on 8 TRN2 NeuronCores — data-parallel over tokens.

Math: reference computes
    gamma_w = mean|W| + eps;  bw = clip(round(W/gamma_w), -1, 1)
    xn = LayerNorm(x);  gamma = max|xn|;  xq = clip(xn*QB/gamma, +-(QB-eps))
    y  = (xq @ bw.T) * (gamma*beta/QB),  beta = max_d sum_o |W[o,d]|
The gamma factor cancels algebraically (clip only nudges the max element
by 1e-5/127 ~ 8e-8 relative), so on device we compute
    y = (LayerNorm(x) @ bw.T) * beta
with NO cross-core collective (verified 6.6e-11 rel err vs reference in
f64).  bw is ternarized with a sign LUT: bw2 = sign(W-thr) + sign(W+thr)
in {-2,0,2} (thr = gamma_w/2), the factor 2 folded into the beta scale.

LayerNorm is folded into the matmul epilogue so the main matmul can
start while inputs are still streaming in:
    y[t,o] = rstd[t]*beta' * ( sum_d xb[d,t]*bw2[d,o] - mu[t]*colsum[o] )
The -mu*colsum rank-1 term is ONE extra matmul accumulated into the same
PSUM group (lhsT = row0-padded -mu, rhs = replicated colsum rows), and
rstd[t]*beta' is a per-token column scalar applied by the ScalarE
PSUM->SBUF epilogue copy.

Layout trick: host passes x and W pre-transposed (contraction dim d on
partitions), so both matmul operands and the output are in natural
layouts and the kernel needs zero on-device transposes.  Per-token
LN statistics over d(=partitions) come from an all-ones stationary
matmul, which also broadcasts results to all partitions for free; the
token-indexed scalars are columnized via a tiny DRAM gather round-trip.
"""

import os
import sys

import numpy as np

for _p in ("/opt/trn_rl_repo", "/root/.axon_site/_ro/trn_rl_repo"):
    if os.path.isdir(_p) and _p not in sys.path:
        sys.path.append(_p)

from concourse import bacc, bass_isa, mybir, tile  # noqa: E402
from concourse.bass_utils import run_bass_kernel_spmd  # noqa: E402

P = 128
D = 2048  # contraction (hidden) dim
O = 2048  # output dim
N_CORES = 8
N_TOK = 4 * 4096
TOK = N_TOK // N_CORES  # tokens per core
KT = D // P  # 16 contraction tiles
MT = TOK // P  # 16 token tiles per core
CH = 512  # psum free chunk (one bank of f32)
NCH = O // CH
EPS = 1e-5
F32 = mybir.dt.float32
BF16 = mybir.dt.bfloat16


def build_nc():
    nc = bacc.Bacc(None, target_bir_lowering=False, debug=False)
    xt = nc.declare_dram_parameter("xt", [D, TOK], F32, isOutput=False)
    fwt = nc.declare_dram_parameter("fwt", [D, O], F32, isOutput=False)
    y = nc.declare_dram_parameter("y", [TOK, O], F32, isOutput=True)

    Alu = mybir.AluOpType
    Act = mybir.ActivationFunctionType
    Ax = mybir.AxisListType

    with tile.TileContext(nc) as tc:
        with (
            tc.tile_pool(name="const", bufs=1) as const,
            tc.tile_pool(name="wpool", bufs=3) as wpool,
            tc.tile_pool(name="bneg", bufs=1) as bnegp,
            tc.tile_pool(name="bw", bufs=KT) as bwp,
            tc.tile_pool(name="xpool", bufs=2) as xpool,
            tc.tile_pool(name="xb", bufs=KT) as xbp,
            tc.tile_pool(name="sq", bufs=2) as sqp,
            tc.tile_pool(name="stt", bufs=4) as stt,
            tc.tile_pool(name="rows", bufs=1) as rows,
            tc.tile_pool(name="ypool", bufs=2) as ypool,
            tc.tile_pool(name="dram", bufs=1, space="DRAM") as dpool,
            tc.tile_pool(name="psum", bufs=8, space="PSUM") as psum,
        ):
            ones_b = const.tile([P, P], BF16)
            nc.vector.memset(ones_b, 1.0)
            eps_t = const.tile([P, 1], F32)
            nc.vector.memset(eps_t, EPS)
            scal = const.tile([P, 8], F32)  # columns: scalar registry

            # ---- W pass A: per-d-row sums of |W| over o ----------------
            wsum = const.tile([P, KT], F32)
            for i in range(KT):
                wa = wpool.tile([P, O], F32, tag="w")
                nc.sync.dma_start(wa, fwt[P * i : P * (i + 1), :])
                nc.vector.tensor_reduce(
                    wsum[:, i : i + 1], wa, axis=Ax.X, op=Alu.add,
                    apply_absolute_value=True,
                )

            row_tot = scal[:, 0:1]  # per-partition total of |W|
            nc.vector.tensor_reduce(row_tot, wsum, axis=Ax.X, op=Alu.add)
            beta_pp = scal[:, 1:2]  # per-partition max row-sum
            nc.vector.tensor_reduce(beta_pp, wsum, axis=Ax.X, op=Alu.max)
            tot_b = scal[:, 2:3]
            nc.gpsimd.partition_all_reduce(
                tot_b, row_tot, channels=P, reduce_op=bass_isa.ReduceOp.add
            )
            beta_b = scal[:, 3:4]
            nc.gpsimd.partition_all_reduce(
                beta_b, beta_pp, channels=P, reduce_op=bass_isa.ReduceOp.max
            )
            # thr = 0.5*gamma_w = 0.5*(tot/(D*O) + EPS)
            thr = scal[:, 4:5]
            nc.scalar.activation(
                thr, tot_b, Act.Copy, bias=0.5 * EPS, scale=0.5 / (D * O)
            )
            nthr = scal[:, 5:6]
            nc.scalar.activation(
                nthr, tot_b, Act.Copy, bias=-0.5 * EPS, scale=-0.5 / (D * O)
            )
            beta_h = scal[:, 6:7]  # beta/2 (bw carries a factor of 2)
            nc.scalar.activation(beta_h, beta_b, Act.Copy, bias=0.0, scale=0.5)

            # ---- W pass B: ternarize via sign LUT ----------------------
            # bw2 = sign(W - thr) + sign(W + thr) in {-2, 0, +2}
            bwts = []
            for i in range(KT):
                wb = wpool.tile([P, O], F32, tag="w")
                nc.sync.dma_start(wb, fwt[P * i : P * (i + 1), :])
                bw = bwp.tile([P, O], BF16, tag="bw")
                nc.scalar.activation(bw, wb, Act.Sign, bias=nthr)
                bneg = bnegp.tile([P, O], BF16, tag="bneg")
                nc.vector.tensor_scalar(
                    out=bneg, in0=wb, scalar1=nthr, scalar2=-2.0,
                    op0=Alu.is_le, op1=Alu.mult,
                )
                # stored weights = bw2 - 1 in {1,-1,-3}; the uniform -1
                # offset cancels exactly through the -mu*colsum correction
                nc.vector.tensor_tensor(out=bw, in0=bw, in1=bneg, op=Alu.add)
                bwts.append(bw)

            # ---- X ingest + LN stats (colsums via ones-matmul) ---------
            xbs = []
            ps_mu = [psum.tile([P, CH], F32, tag="ps", name=f"ps_mu{c}") for c in range(NCH)]
            ps_sq = [psum.tile([P, CH], F32, tag="ps", name=f"ps_sq{c}") for c in range(NCH)]
            for k in range(KT):
                xk = xpool.tile([P, TOK], F32, tag="x")
                nc.sync.dma_start(xk, xt[P * k : P * (k + 1), :])
                xb = xbp.tile([P, TOK], BF16, tag="xb")
                nc.vector.tensor_copy(out=xb, in_=xk)
                xbs.append(xb)
                first, last = k == 0, k == KT - 1
                for c in range(NCH):
                    sl = slice(CH * c, CH * (c + 1))
                    xsq = sqp.tile([P, CH], BF16, tag="xsq")
                    if c < NCH // 2:
                        nc.scalar.activation(xsq, xb[:, sl], Act.Square)
                    else:
                        nc.vector.tensor_tensor(
                            out=xsq, in0=xb[:, sl], in1=xb[:, sl], op=Alu.mult
                        )
                    nc.tensor.matmul(
                        ps_mu[c], ones_b, xb[:, sl], start=first, stop=last
                    )
                    nc.tensor.matmul(ps_sq[c], ones_b, xsq, start=first, stop=last)

            # ---- colsum of bw2 over d (for the -mu*colsum correction) --
            ps_cs = [psum.tile([P, CH], F32, tag="ps", name=f"ps_cs{c}") for c in range(NCH)]
            for k in range(KT):
                first, last = k == 0, k == KT - 1
                for c in range(NCH):
                    nc.tensor.matmul(
                        ps_cs[c], ones_b, bwts[k][:, CH * c : CH * (c + 1)],
                        start=first, stop=last,
                    )
            crep = []
            for c in range(NCH):
                ct = rows.tile([P, CH], BF16, tag=f"crep{c}")
                nc.vector.memset(ct, 0.0)
                nc.vector.memset(ct[0:2, :], -2048.0)
                nc.scalar.activation(
                    ct[0:1, :], ps_cs[c][0:1, :], Act.Copy, bias=2048.0
                )
                crep.append(ct)

            # ---- LN stats finalize:
            #   negmu row0 = -mu (rest 0), rb_row = rstd*beta/2 ----------
            negmu = rows.tile([P, TOK], BF16, tag="negmu")
            nc.vector.memset(negmu, 0.0)
            rb_row = rows.tile([1, TOK], F32, tag="rb_row")
            for c in range(NCH):
                sl = slice(CH * c, CH * (c + 1))
                mu_c = stt.tile([P, CH], F32, tag="stt")
                nc.scalar.mul(mu_c, ps_mu[c], 1.0 / D)
                var_c = stt.tile([P, CH], F32, tag="stt")
                nc.scalar.mul(var_c, ps_sq[c], 1.0 / D)  # E[x^2]
                nc.scalar.activation(
                    negmu[0:2, sl], mu_c[0:2, :], Act.Copy, bias=0.0, scale=-1.0
                )
                musq = stt.tile([P, CH], F32, tag="stt")
                nc.scalar.activation(musq, mu_c, Act.Square)
                nc.vector.tensor_tensor(
                    out=var_c, in0=var_c, in1=musq, op=Alu.subtract
                )
                nc.scalar.activation(var_c, var_c, Act.Sqrt, bias=eps_t)
                rstd_c = stt.tile([P, CH], F32, tag="stt")
                nc.vector.reciprocal(rstd_c, var_c)
                nc.vector.tensor_scalar(
                    out=rb_row[0:1, sl], in0=rstd_c[0:1, :],
                    scalar1=beta_h[0:1, 0:1], scalar2=None, op0=Alu.mult,
                )

            # columnize rb_row: [1, TOK] -> [P, MT] via DRAM gather ------
            rb_dram = dpool.tile([TOK], F32)
            nc.sync.dma_start(rb_dram[None, :], rb_row)
            rb_col = const.tile([P, MT], F32)
            with nc.allow_non_contiguous_dma(reason="2048x4B one-time gather"):
                nc.sync.dma_start(rb_col, rb_dram.rearrange("(m p) -> p m", p=P))

            # ---- main matmul + fused LN epilogue -----------------------
            for m in range(MT):
                pys = [psum.tile([P, CH], F32, tag="ps", name=f"py{m}_{c}") for c in range(NCH)]
                for k in range(KT):
                    lhs = xbs[k][:, P * m : P * (m + 1)]
                    first = k == 0
                    for c in range(NCH):
                        nc.tensor.matmul(
                            pys[c],
                            lhs,
                            bwts[k][:, CH * c : CH * (c + 1)],
                            start=first,
                            stop=False,
                        )
                # rank-1 correction: psum += (-mu[t]) * colsum[o]
                nmslice = negmu[:, P * m : P * (m + 1)]
                for c in range(NCH):
                    nc.tensor.matmul(pys[c], nmslice, crep[c], start=False, stop=True)
                for h in range(2):
                    ysb = ypool.tile([P, O // 2], F32, tag="y")
                    for cc in range(NCH // 2):
                        c = h * (NCH // 2) + cc
                        nc.scalar.mul(
                            ysb[:, CH * cc : CH * (cc + 1)], pys[c],
                            rb_col[:, m : m + 1],
                        )
                    nc.sync.dma_start(
                        y[P * m : P * (m + 1), h * (O // 2) : (h + 1) * (O // 2)],
                        ysb,
                    )

    nc.compile()
    return nc


_NC_CACHE = None


def _get_nc():
    global _NC_CACHE
    if _NC_CACHE is None:
        _NC_CACHE = build_nc()
    return _NC_CACHE


def _prep_in_maps(x, fweight):
    x2 = np.ascontiguousarray(x, dtype=np.float32).reshape(N_TOK, D)
    fwt = np.ascontiguousarray(np.asarray(fweight, dtype=np.float32).T)
    in_maps = []
    for c in range(N_CORES):
        xs = np.ascontiguousarray(x2[c * TOK : (c + 1) * TOK, :].T)
        in_maps.append({"xt": xs, "fwt": fwt})
    return in_maps


def run_spmd(x, fweight, **kw):
    nc = _get_nc()
    in_maps = _prep_in_maps(x, fweight)
    return run_bass_kernel_spmd(nc, in_maps, core_ids=list(range(N_CORES)), **kw)


def kernel(x, fweight):
    res = run_spmd(x, fweight)
    y = np.concatenate([res.results[c]["y"] for c in range(N_CORES)], axis=0)
    return y.reshape(4, 4096, O)


if __name__ == "__main__":
    xx = np.random.randn(4, 4096, D).astype(np.float32)
    ww = np.random.uniform(-1 / np.sqrt(D), 1 / np.sqrt(D), (O, D)).astype(np.float32)
    out = kernel(xx, ww)
    print("out", out.shape, out.dtype, float(np.abs(out).mean()))


# revision 17
# speedup vs baseline: 1.0958x; 1.0958x over previous
"""BitLinear forward on 8 TRN2 NeuronCores — data-parallel over tokens.

Math: reference computes
    gamma_w = mean|W| + eps;  bw = clip(round(W/gamma_w), -1, 1)
    xn = LayerNorm(x);  gamma = max|xn|;  xq = clip(xn*QB/gamma, +-(QB-eps))
    y  = (xq @ bw.T) * (gamma*beta/QB),  beta = max_d sum_o |W[o,d]|
The gamma factor cancels algebraically (clip only nudges the max element
by 1e-5/127 ~ 8e-8 relative), so on device we compute
    y = (LayerNorm(x) @ bw.T) * beta
with NO cross-core collective (verified 6.6e-11 rel err vs reference in
f64).  bw is ternarized with a sign LUT: bw2 = sign(W-thr) + sign(W+thr)
in {-2,0,2} (thr = gamma_w/2), the factor 2 folded into the beta scale.

LayerNorm is folded into the matmul epilogue so the main matmul can
start while inputs are still streaming in:
    y[t,o] = rstd[t]*beta' * ( sum_d xb[d,t]*bw2[d,o] - mu[t]*colsum[o] )
The -mu*colsum rank-1 term is ONE extra matmul accumulated into the same
PSUM group (lhsT = row0-padded -mu, rhs = replicated colsum rows), and
rstd[t]*beta' is a per-token column scalar applied by the ScalarE
PSUM->SBUF epilogue copy.

Layout trick: host passes x and W pre-transposed (contraction dim d on
partitions), so both matmul operands and the output are in natural
layouts and the kernel needs zero on-device transposes.  Per-token
LN statistics over d(=partitions) come from an all-ones stationary
matmul, which also broadcasts results to all partitions for free; the
token-indexed scalars are columnized via a tiny DRAM gather round-trip.
"""

import os
import sys

import numpy as np

for _p in ("/opt/trn_rl_repo", "/root/.axon_site/_ro/trn_rl_repo"):
    if os.path.isdir(_p) and _p not in sys.path:
        sys.path.append(_p)

from concourse import bacc, bass_isa, mybir, tile  # noqa: E402
from concourse.bass_utils import run_bass_kernel_spmd  # noqa: E402

P = 128
D = 2048  # contraction (hidden) dim
O = 2048  # output dim
N_CORES = 8
N_TOK = 4 * 4096
TOK = N_TOK // N_CORES  # tokens per core
KT = D // P  # 16 contraction tiles
MT = TOK // P  # 16 token tiles per core
CH = 512  # psum free chunk (one bank of f32)
NCH = O // CH
EPS = 1e-5
F32 = mybir.dt.float32
BF16 = mybir.dt.bfloat16


def build_nc():
    nc = bacc.Bacc(None, target_bir_lowering=False, debug=False)
    xt = nc.declare_dram_parameter("xt", [D, TOK], F32, isOutput=False)
    fwt = nc.declare_dram_parameter("fwt", [D, O], F32, isOutput=False)
    y = nc.declare_dram_parameter("y", [TOK, O], F32, isOutput=True)

    Alu = mybir.AluOpType
    Act = mybir.ActivationFunctionType
    Ax = mybir.AxisListType

    with tile.TileContext(nc) as tc:
        with (
            tc.tile_pool(name="const", bufs=1) as const,
            tc.tile_pool(name="wpool", bufs=2) as wpool,
            tc.tile_pool(name="bneg", bufs=2) as bnegp,
            tc.tile_pool(name="bw", bufs=KT) as bwp,
            tc.tile_pool(name="xpool", bufs=2) as xpool,
            tc.tile_pool(name="xb", bufs=KT) as xbp,
            tc.tile_pool(name="sq", bufs=2) as sqp,
            tc.tile_pool(name="stt", bufs=4) as stt,
            tc.tile_pool(name="rows", bufs=1) as rows,
            tc.tile_pool(name="ypool", bufs=2) as ypool,
            tc.tile_pool(name="dram", bufs=1, space="DRAM") as dpool,
            tc.tile_pool(name="psum", bufs=8, space="PSUM") as psum,
        ):
            ones_b = const.tile([P, P], BF16)
            nc.vector.memset(ones_b, 1.0)
            eps_t = const.tile([P, 1], F32)
            nc.vector.memset(eps_t, EPS)
            scal = const.tile([P, 8], F32)  # columns: scalar registry

            # ---- W pass A: per-d-row sums of |W| over o ----------------
            wsum = const.tile([P, KT], F32)
            for i in range(KT):
                wa = wpool.tile([P, O], F32, tag="w")
                nc.sync.dma_start(wa, fwt[P * i : P * (i + 1), :])
                nc.vector.tensor_reduce(
                    wsum[:, i : i + 1], wa, axis=Ax.X, op=Alu.add,
                    apply_absolute_value=True,
                )

            row_tot = scal[:, 0:1]  # per-partition total of |W|
            nc.vector.tensor_reduce(row_tot, wsum, axis=Ax.X, op=Alu.add)
            beta_pp = scal[:, 1:2]  # per-partition max row-sum
            nc.vector.tensor_reduce(beta_pp, wsum, axis=Ax.X, op=Alu.max)
            tot_b = scal[:, 2:3]
            nc.gpsimd.partition_all_reduce(
                tot_b, row_tot, channels=P, reduce_op=bass_isa.ReduceOp.add
            )
            beta_b = scal[:, 3:4]
            nc.gpsimd.partition_all_reduce(
                beta_b, beta_pp, channels=P, reduce_op=bass_isa.ReduceOp.max
            )
            # thr = 0.5*gamma_w = 0.5*(tot/(D*O) + EPS)
            thr = scal[:, 4:5]
            nc.scalar.activation(
                thr, tot_b, Act.Copy, bias=0.5 * EPS, scale=0.5 / (D * O)
            )
            nthr = scal[:, 5:6]
            nc.scalar.activation(
                nthr, tot_b, Act.Copy, bias=-0.5 * EPS, scale=-0.5 / (D * O)
            )
            beta_h = scal[:, 6:7]  # beta/2 (bw carries a factor of 2)
            nc.scalar.activation(beta_h, beta_b, Act.Copy, bias=0.0, scale=0.5)

            # ---- W pass B: ternarize via sign LUT ----------------------
            # bw2 = sign(W - thr) + sign(W + thr) in {-2, 0, +2}
            bwts = []
            for i in range(KT):
                wb = wpool.tile([P, O], F32, tag="w")
                nc.sync.dma_start(wb, fwt[P * i : P * (i + 1), :])
                bw = bwp.tile([P, O], BF16, tag="bw")
                nc.scalar.activation(bw, wb, Act.Sign, bias=nthr)
                bneg = bnegp.tile([P, O], BF16, tag="bneg")
                nc.vector.tensor_scalar(
                    out=bneg, in0=wb, scalar1=nthr, scalar2=-2.0,
                    op0=Alu.is_le, op1=Alu.mult,
                )
                # stored weights = bw2 - 1 in {1,-1,-3}; the uniform -1
                # offset cancels exactly through the -mu*colsum correction
                nc.vector.tensor_tensor(out=bw, in0=bw, in1=bneg, op=Alu.add)
                bwts.append(bw)

            # ---- X ingest + LN stats (colsums via ones-matmul) ---------
            xbs = []
            ps_mu = [psum.tile([P, CH], F32, tag="ps", name=f"ps_mu{c}") for c in range(NCH)]
            ps_sq = [psum.tile([P, CH], F32, tag="ps", name=f"ps_sq{c}") for c in range(NCH)]
            for k in range(KT):
                xk = xpool.tile([P, TOK], F32, tag="x")
                nc.sync.dma_start(xk, xt[P * k : P * (k + 1), :])
                xb = xbp.tile([P, TOK], BF16, tag="xb")
                nc.vector.tensor_copy(out=xb, in_=xk)
                xbs.append(xb)
                first, last = k == 0, k == KT - 1
                for c in range(NCH):
                    sl = slice(CH * c, CH * (c + 1))
                    xsq = sqp.tile([P, CH], BF16, tag="xsq")
                    if c < NCH // 2:
                        nc.scalar.activation(xsq, xb[:, sl], Act.Square)
                    else:
                        nc.vector.tensor_tensor(
                            out=xsq, in0=xb[:, sl], in1=xb[:, sl], op=Alu.mult
                        )
                    nc.tensor.matmul(
                        ps_mu[c], ones_b, xb[:, sl], start=first, stop=last
                    )
                    nc.tensor.matmul(ps_sq[c], ones_b, xsq, start=first, stop=last)

            # ---- colsum of bw2 over d (for the -mu*colsum correction) --
            ps_cs = [psum.tile([P, CH], F32, tag="ps", name=f"ps_cs{c}") for c in range(NCH)]
            for k in range(KT):
                first, last = k == 0, k == KT - 1
                for c in range(NCH):
                    nc.tensor.matmul(
                        ps_cs[c], ones_b, bwts[k][:, CH * c : CH * (c + 1)],
                        start=first, stop=last,
                    )
            crep = []
            for c in range(NCH):
                ct = rows.tile([P, CH], BF16, tag=f"crep{c}")
                nc.vector.memset(ct, 0.0)
                nc.vector.memset(ct[0:2, :], -2048.0)
                nc.scalar.activation(
                    ct[0:1, :], ps_cs[c][0:1, :], Act.Copy, bias=2048.0
                )
                crep.append(ct)

            # ---- LN stats finalize:
            #   negmu row0 = -mu (rest 0), rb_row = rstd*beta/2 ----------
            negmu = rows.tile([P, TOK], BF16, tag="negmu")
            nc.vector.memset(negmu, 0.0)
            rb_row = rows.tile([1, TOK], F32, tag="rb_row")
            for c in range(NCH):
                sl = slice(CH * c, CH * (c + 1))
                mu_c = stt.tile([P, CH], F32, tag="stt")
                nc.scalar.mul(mu_c, ps_mu[c], 1.0 / D)
                var_c = stt.tile([P, CH], F32, tag="stt")
                nc.scalar.mul(var_c, ps_sq[c], 1.0 / D)  # E[x^2]
                nc.scalar.activation(
                    negmu[0:2, sl], mu_c[0:2, :], Act.Copy, bias=0.0, scale=-1.0
                )
                musq = stt.tile([P, CH], F32, tag="stt")
                nc.scalar.activation(musq, mu_c, Act.Square)
                nc.vector.tensor_tensor(
                    out=var_c, in0=var_c, in1=musq, op=Alu.subtract
                )
                nc.scalar.activation(var_c, var_c, Act.Sqrt, bias=eps_t)
                rstd_c = stt.tile([P, CH], F32, tag="stt")
                nc.vector.reciprocal(rstd_c, var_c)
                nc.vector.tensor_scalar(
                    out=rb_row[0:1, sl], in0=rstd_c[0:1, :],
                    scalar1=beta_h[0:1, 0:1], scalar2=None, op0=Alu.mult,
                )

            # columnize rb_row: [1, TOK] -> [P, MT] via DRAM gather ------
            rb_dram = dpool.tile([TOK], F32)
            nc.sync.dma_start(rb_dram[None, :], rb_row)
            rb_col = const.tile([P, MT], F32)
            with nc.allow_non_contiguous_dma(reason="2048x4B one-time gather"):
                nc.sync.dma_start(rb_col, rb_dram.rearrange("(m p) -> p m", p=P))

            # ---- main matmul + fused LN epilogue -----------------------
            for m in range(MT):
                pys = [psum.tile([P, CH], F32, tag="ps", name=f"py{m}_{c}") for c in range(NCH)]
                for k in range(KT):
                    lhs = xbs[k][:, P * m : P * (m + 1)]
                    first = k == 0
                    for c in range(NCH):
                        nc.tensor.matmul(
                            pys[c],
                            lhs,
                            bwts[k][:, CH * c : CH * (c + 1)],
                            start=first,
                            stop=False,
                        )
                # rank-1 correction: psum += (-mu[t]) * colsum[o]
                nmslice = negmu[:, P * m : P * (m + 1)]
                for c in range(NCH):
                    nc.tensor.matmul(pys[c], nmslice, crep[c], start=False, stop=True)
                for h in range(2):
                    ysb = ypool.tile([P, O // 2], F32, tag="y")
                    for cc in range(NCH // 2):
                        c = h * (NCH // 2) + cc
                        nc.scalar.mul(
                            ysb[:, CH * cc : CH * (cc + 1)], pys[c],
                            rb_col[:, m : m + 1],
                        )
                    nc.sync.dma_start(
                        y[P * m : P * (m + 1), h * (O // 2) : (h + 1) * (O // 2)],
                        ysb,
                    )

    nc.compile()
    return nc


_NC_CACHE = None


def _get_nc():
    global _NC_CACHE
    if _NC_CACHE is None:
        _NC_CACHE = build_nc()
    return _NC_CACHE


def _prep_in_maps(x, fweight):
    x2 = np.ascontiguousarray(x, dtype=np.float32).reshape(N_TOK, D)
    fwt = np.ascontiguousarray(np.asarray(fweight, dtype=np.float32).T)
    in_maps = []
    for c in range(N_CORES):
        xs = np.ascontiguousarray(x2[c * TOK : (c + 1) * TOK, :].T)
        in_maps.append({"xt": xs, "fwt": fwt})
    return in_maps


def run_spmd(x, fweight, **kw):
    nc = _get_nc()
    in_maps = _prep_in_maps(x, fweight)
    return run_bass_kernel_spmd(nc, in_maps, core_ids=list(range(N_CORES)), **kw)


def kernel(x, fweight):
    res = run_spmd(x, fweight)
    y = np.concatenate([res.results[c]["y"] for c in range(N_CORES)], axis=0)
    return y.reshape(4, 4096, O)


if __name__ == "__main__":
    xx = np.random.randn(4, 4096, D).astype(np.float32)
    ww = np.random.uniform(-1 / np.sqrt(D), 1 / np.sqrt(D), (O, D)).astype(np.float32)
    out = kernel(xx, ww)
    print("out", out.shape, out.dtype, float(np.abs(out).mean()))
